# revision 1
# baseline (speedup 1.0000x reference)
"""BallQueryAttention TRN2 kernel.

Math: reference computes softmax over a binary ball mask (d2 <= R^2), then
mask-softmax @ x.  exp of a 0/1 mask takes only values {1, e}, so

  out[i] = (S + (e-1) * sum_{j in ball(i)} x_j) / (N + (e-1) * cnt_i)

with S = colsum(x).  Sharding: rows (i) across 8 cores, x replicated.

Per core (row shard of 1024):
  - Gram tiles Gt[j_tile(128 part), i(1024 free)] via 2 fp16 hi/lo-split
    matmuls (fp32-class accuracy on the distance threshold).  The sq_i term
    rides 3 fp16-split augmentation rows on the moving operand; the sq_j
    term rides fp32 per-partition bias/threshold in the compare op.
  - mask compare split across Vector (is_ge -> {0,2}) and Scalar
    (Sign -> {-1,1}) engines, writing fp16 masks.
  - accumulating [x|1]^T @ mask matmul -> [65, 1024] in PSUM, plus
    ones-column sums (SALL over all tiles, SAO over sign tiles) to undo
    the {0,2}/{-1,1} conventions:
      numer/denom = SALL + K1*(OUT2 + SAO),  K1 = (e-1)/2
  - PE transpose + reciprocal + per-partition scale for the final divide.

Transposed fp16 layouts (d-on-partitions) are produced by DMA-transpose of
[rows, 128] fp16 DRAM scratch ([hi|lo] packed), pipelined in 8 column
groups so the PE starts after ~1/8 of the preamble.
"""

import sys

sys.path.insert(0, "/opt/trn_rl_repo")

import numpy as np

import concourse.bass as bass
import concourse.tile as tile
from concourse import bacc, masks, mybir
from concourse.bass_utils import run_bass_kernel_spmd

F32 = mybir.dt.float32
F16 = mybir.dt.float16
AF = mybir.ActivationFunctionType
OP = mybir.AluOpType

N = 8192
D = 64
NCORES = 8
ROWS = N // NCORES          # 1024 rows per core
JT = N // 128               # 64 j-tiles
IT = ROWS // 128            # 8 i-tiles
NG = 16                     # preamble column groups
TPG = JT // NG              # j-tiles per group
R2 = 11.0 * 11.0
K1 = (np.e - 1.0) / 2.0

# experiment flags (consulted at module-build time)
FLAGS = {
    "xw": True,        # build XW
    "hilo": True,      # hi/lo staging + hilod write
    "trans": True,     # W1/W2 transposes
    "sq": True,        # square/reduce/bias
    "iside": True,     # i-side preamble
    "compare": True,   # real compare (False -> memset masks once)
    "sall": True,      # SALL matmuls in loop
    "passc": True,     # pass C matmuls
    "gmm": True,       # G matmuls
    "lag": 2,
}


def _calib_body(nc, tc, pools):
    const, scratch, gpool, mpool, apool, spool = pools
    Gh = gpool.tile([128, 512], mybir.dt.float32, tag="G")
    onec = const.tile([128, 1], F16, tag="calib_one")
    nc.vector.memset(onec[:], 1.0)
    nc.tensor.matmul(Gh[0:1, 0:1], onec[:], onec[:], start=True, stop=True)


def _pre(nc, tc, pools, xf, xi, outd, dram):
    const, scratch, gpool, mpool, apool, spool = pools
    ts = bass.ts

    # ---------------- persistent tiles ----------------
    W1g = [const.tile([128, TPG * 128], F16, name=f"W1_{g}", tag=f"W1_{g}")
           for g in range(NG)]
    ONES128 = const.tile([128, 128], F16, tag="ONES128")
    XW = const.tile([128, 65 * JT], F16, tag="XW")
    R1 = const.tile([128, ROWS], F16, tag="R1")     # [hiT_i; hiT_i]
    R2t = const.tile([128, ROWS], F16, tag="R2")    # [loT_i; v1; v2; v3; 0]
    biasA = const.tile([128, JT], F32, tag="biasA")
    thrD = const.tile([128, JT], F32, tag="thrD")
    ONEC = const.tile([128, 1], F16, tag="ONEC")
    IDN = const.tile([65, 65], F32, tag="IDN")

    # ---------------- preamble: i side (small) ----------------
    r1d = dram.tile([ROWS, 128], F16, tag="r1d")
    r2d = dram.tile([ROWS, 128], F16, tag="r2d")

    if not FLAGS["iside"]:
        nc.vector.memset(R1[:], 0.0)
        nc.vector.memset(R2t[:], 0.0)
        nc.vector.memset(ONEC[:], 1.0)
        nc.vector.memset(ONES128[:], 1.0)
        masks.make_identity(nc, IDN[:])
        _j_side(nc, tc, pools, xf, dram, W1g, XW, biasA, thrD)
        return dict(W1g=W1g, ONES128=ONES128, XW=XW, R1=R1, R2t=R2t,
                    biasA=biasA, thrD=thrD, ONEC=ONEC, IDN=IDN)

    xitp = scratch.tile([128, IT * D], F32, tag="xitp")  # row p*IT + t
    nc.gpsimd.dma_start(xitp[:], xi.rearrange("(p t) d -> p (t d)", p=128))
    xitp3 = xitp[:].rearrange("p (t d) -> p t d", d=D)

    r1sb = scratch.tile([128, IT * 128], F16, tag="r1sb")
    r13 = r1sb[:].rearrange("p (t e) -> p t e", e=128)
    nc.vector.tensor_copy(r13[:, :, 0:D], xitp3)          # hi_i
    r2sb = scratch.tile([128, IT * 128], F16, tag="r2sb")
    r23 = r2sb[:].rearrange("p (t e) -> p t e", e=128)
    nc.gpsimd.memset(r23[:, :, D:128], 0.0)
    nc.vector.tensor_tensor(r23[:, :, 0:D], xitp3, r13[:, :, 0:D], OP.subtract)  # lo_i
    nc.vector.tensor_copy(r13[:, :, D : 2 * D], r13[:, :, 0:D])  # dup hi_i

    s2i = scratch.tile([128, IT * D], F32, tag="s2i")
    nc.scalar.activation(s2i[:], xitp[:], AF.Square)
    sqit = scratch.tile([128, IT], F32, tag="sqit")
    nc.vector.tensor_reduce(sqit[:], s2i[:].rearrange("p (t d) -> p t d", d=D),
                            axis=mybir.AxisListType.X, op=OP.add)
    vfull = scratch.tile([128, IT], F32, tag="vfull")
    nc.vector.tensor_scalar(vfull[:], sqit[:], -0.5, None, OP.mult)
    v1 = scratch.tile([128, IT], F16, tag="v1")
    nc.vector.tensor_copy(v1[:], vfull[:])
    rv1 = scratch.tile([128, IT], F32, tag="rv1")
    nc.vector.tensor_tensor(rv1[:], vfull[:], v1[:], OP.subtract)
    v2 = scratch.tile([128, IT], F16, tag="v2")
    nc.vector.tensor_copy(v2[:], rv1[:])
    rv2 = scratch.tile([128, IT], F32, tag="rv2")
    nc.vector.tensor_tensor(rv2[:], rv1[:], v2[:], OP.subtract)
    v3 = scratch.tile([128, IT], F16, tag="v3")
    nc.vector.tensor_copy(v3[:], rv2[:])
    for k, vk in enumerate((v1, v2, v3)):
        nc.vector.tensor_copy(
            r23[:, :, D + k : D + k + 1],
            vk[:].rearrange("p (t u) -> p t u", u=1),
        )
    nc.gpsimd.dma_start(r1d[:].rearrange("(p t) e -> p (t e)", p=128), r1sb[:])
    nc.gpsimd.dma_start(r2d[:].rearrange("(p t) e -> p (t e)", p=128), r2sb[:])
    nc.sync.dma_start(R1[:], r1d[:], transpose=True)
    nc.sync.dma_start(R2t[:], r2d[:], transpose=True)

    nc.vector.memset(ONEC[:], 1.0)
    nc.vector.memset(ONES128[:], 1.0)
    masks.make_identity(nc, IDN[:])

    # trigger the Sign act-table load early so it overlaps the preamble
    dumm = spool.tile([128, 1], F32, tag="dumm")
    nc.scalar.activation(dumm[:], xitp[:, 0:1], AF.Sign)

    _j_side(nc, tc, pools, xf, dram, W1g, XW, biasA, thrD)

    return dict(W1g=W1g, ONES128=ONES128, XW=XW, R1=R1, R2t=R2t, biasA=biasA,
                thrD=thrD, ONEC=ONEC, IDN=IDN)


def _j_side(nc, tc, pools, xf, dram, W1g, XW, biasA, thrD):
    const, scratch, gpool, mpool, apool, spool = pools
    ts = bass.ts
    hilod = dram.tile([N, 128], F16, tag="hilod")
    # -------- preamble: j side. Phase 1 (copies, ACT ring) --------
    for g in range(NG):
        rows = TPG * 128  # rows covered by this group
        xtp = scratch.tile([128, TPG * D], F32, tag="xtp")  # bufs>1 pool
        # j-tile t holds rows {c*64+t : c in 0..127}; this makes the x load
        # contiguous per partition (the j dimension is only ever summed over,
        # so the relabeling is invisible outside)
        nc.gpsimd.dma_start(
            xtp[:].rearrange("p (t d) -> p t d", d=D),
            xf.rearrange("(p t) d -> p t d", p=128)[:, g * TPG : (g + 1) * TPG, :],
        )
        xtp3 = xtp[:].rearrange("p (t d) -> p t d", d=D)

        # XW slice for this group
        if FLAGS["xw"]:
            xw3 = XW[:].rearrange("p (t e) -> p t e", e=65)[:, g * TPG : (g + 1) * TPG, :]
            nc.vector.tensor_copy(xw3[:, :, 0:D], xtp3)
            nc.gpsimd.memset(xw3[:, :, D : D + 1], 1.0)

        # hi/lo staging -> hilod_g -> two transposes
        if FLAGS["hilo"]:
            hilo = scratch.tile([128, TPG * 128], F16, tag="hilo")
            hl3 = hilo[:].rearrange("p (t e) -> p t e", e=128)
            nc.vector.tensor_copy(hl3[:, :, 0:D], xtp3)
            nc.vector.tensor_tensor(hl3[:, :, D : 2 * D], xtp3, hl3[:, :, 0:D],
                                    OP.subtract)
            hseg = hilod[g * rows : (g + 1) * rows, :]
            nc.gpsimd.dma_start(hseg.rearrange("(t p) e -> p t e", p=128), hl3)
            if FLAGS["trans"]:
                eng = nc.sync if g % 2 == 0 else nc.scalar
                eng.dma_start(W1g[g][:], hseg, transpose=True)

        # sq_j -> bias/thr columns for this group's tiles
        if FLAGS["sq"]:
            s2 = scratch.tile([128, TPG * D], F32, tag="s2")
            nc.scalar.activation(s2[:], xtp[:], AF.Square)
            sl = slice(g * TPG, (g + 1) * TPG)
            nc.vector.tensor_reduce(biasA[:, sl],
                                    s2[:].rearrange("p (t d) -> p t d", d=D),
                                    axis=mybir.AxisListType.X, op=OP.add)
            nc.vector.tensor_scalar(thrD[:, sl], biasA[:, sl], 0.5, -R2 / 2.0,
                                    OP.mult, OP.add)
            nc.vector.tensor_scalar(biasA[:, sl], biasA[:, sl], -0.5, R2 / 2.0,
                                    OP.mult, OP.add)



def _main(nc, tc, pools, outd, env):
    const, scratch, gpool, mpool, apool, spool = pools
    ts = bass.ts
    W1g = env['W1g']; ONES128 = env['ONES128']; XW = env['XW']; R1 = env['R1']
    R2t = env['R2t']; biasA = env['biasA']; thrD = env['thrD']
    ONEC = env['ONEC']; IDN = env['IDN']

    # ---------------- psum accumulators ----------------
    # column block 0:512 always gets the DVE {0,2} mask convention and
    # block 512:1024 the ACT {-1,1} one, so the sign-correction term is
    # just SALL itself:
    #   P[:, 0:512]    = K1*OUT2 + SALL
    #   P[:, 512:1024] = K1*OUT2 + (1+K1)*SALL
    OUT2 = apool.tile([65, ROWS], F32, tag="OUT2")
    SALL = apool.tile([65, 1], F32, tag="SALL")

    # ------- main loop over half j-tiles, pass C lagged by LAG halves ----
    LAG = FLAGS["lag"]
    NH = 2 * JT
    mks = {}
    fixed_mk = None
    if not FLAGS["compare"]:
        fixed_mk = const.tile([128, 512], F16, tag="fixed_mk")
        nc.vector.memset(fixed_mk[:], 1.0)
    for idx in range(NH + LAG):
        if idx < NH:
            t, h = divmod(idx, 2)
            g, tt = divmod(t, TPG)
            cs = slice(512 * h, 512 * (h + 1))
            if FLAGS["gmm"]:
                Gh = gpool.tile([128, 512], F32, tag="G")
                nc.tensor.matmul(Gh[:], W1g[g][:, ts(tt, 128)], R1[:, cs],
                                 start=True, stop=False)
                nc.tensor.matmul(Gh[:], W1g[g][0:64, ts(tt, 128)],
                                 R2t[0:64, cs], start=False, stop=False)
                # v-aug rides PE rows 64-66 concurrently with the pass above
                nc.tensor.matmul(Gh[:], ONES128[64:67, :], R2t[64:67, cs],
                                 start=False, stop=True)
            if FLAGS["compare"]:
                mk = mpool.tile([128, 512], F16, tag="mk")
                if idx % 2 == 0:
                    nc.vector.tensor_scalar(mk[:], Gh[:], thrD[:, t : t + 1],
                                            2.0, OP.is_ge, OP.mult)
                else:
                    nc.scalar.activation(mk[:], Gh[:], AF.Sign,
                                         bias=biasA[:, t : t + 1])
                mks[idx] = mk
            else:
                mks[idx] = fixed_mk
        if idx >= LAG and FLAGS["passc"]:
            jdx = idx - LAG
            t, h = divmod(jdx, 2)
            cs = slice(512 * h, 512 * (h + 1))
            xws = XW[:, 65 * t : 65 * (t + 1)]
            nc.tensor.matmul(OUT2[:, cs], xws, mks.pop(jdx)[:],
                             start=(t == 0), stop=(t == JT - 1))
            if h == 1 and FLAGS["sall"]:
                nc.tensor.matmul(SALL[:], xws, ONEC[:],
                                 start=(t == 0), stop=(t == JT - 1))

    # ---------------- tail (per i-chunk, DVE/ACT alternating) -----------
    sallsb = spool.tile([65, 1], F32, tag="sallsb")
    nc.vector.tensor_copy(sallsb[:], SALL[:])
    b1sb = spool.tile([65, 1], F32, tag="b1sb")
    nc.vector.tensor_scalar(b1sb[:], sallsb[:], 1.0 + K1, None, OP.mult)

    for c in range(IT):
        bap = sallsb if c < IT // 2 else b1sb
        pc = spool.tile([65, 128], F32, tag="pc")
        if c % 2 == 0:
            nc.vector.tensor_scalar(pc[:], OUT2[:, ts(c, 128)], K1, bap[:],
                                    OP.mult, OP.add)
        else:
            nc.scalar.activation(pc[:], OUT2[:, ts(c, 128)], AF.Identity,
                                 bias=bap[:], scale=K1)
        pt = gpool.tile([128, 65], F32, tag="G")
        nc.tensor.transpose(pt[:], pc[:], IDN[:])
        dinv = spool.tile([128, 1], F32, tag="dinv")
        nc.vector.reciprocal(dinv[:], pt[:, D : D + 1])
        ot = spool.tile([128, D], F32, tag="ot")
        nc.vector.tensor_scalar(ot[:], pt[:, 0:D], dinv[:], None, OP.mult)
        nc.sync.dma_start(outd[ts(c, 128), :], ot[:])


def build_module(loop_n=1, scope='full'):
    nc = bacc.Bacc("TRN2", target_bir_lowering=False, debug=False,
                   num_devices=NCORES)
    xf_d = nc.dram_tensor("xf", [N, D], F32, kind="ExternalInput")
    xi_d = nc.dram_tensor("xi", [ROWS, D], F32, kind="ExternalInput")
    out_d = nc.dram_tensor("out", [ROWS, D], F32, kind="ExternalOutput")

    with tile.TileContext(nc) as tc:
        with (
            tc.tile_pool(name="const", bufs=1) as const,
            tc.tile_pool(name="scratch", bufs=2) as scratch,
            tc.tile_pool(name="gpool", bufs=5, space="PSUM") as gpool,
            tc.tile_pool(name="acc", bufs=1, space="PSUM") as apool,
            tc.tile_pool(name="mk", bufs=8) as mpool,
            tc.tile_pool(name="small", bufs=3) as spool,
            tc.tile_pool(name="dram", bufs=3, space="DRAM") as dram,
        ):
            pools = (const, scratch, gpool, mpool, apool, spool)
            args = (nc, tc, pools, xf_d.ap(), xi_d.ap(), out_d.ap(), dram)
            if scope == 'calib':
                with tc.For_i(0, loop_n) as _:
                    _calib_body(nc, tc, pools)
            elif scope == 'pre':
                with tc.For_i(0, loop_n) as _:
                    _pre(*args)
            elif scope == 'main':
                env = _pre(*args)
                with tc.For_i(0, loop_n) as _:
                    _main(nc, tc, pools, out_d.ap(), env)
            elif loop_n == 1:
                env = _pre(*args)
                _main(nc, tc, pools, out_d.ap(), env)
            else:
                with tc.For_i(0, loop_n) as _:
                    env = _pre(*args)
                    _main(nc, tc, pools, out_d.ap(), env)
    nc.finalize()
    return nc


_module_cache = {}


def _get_module(loop_n=1):
    if loop_n not in _module_cache:
        _module_cache[loop_n] = build_module(loop_n)
    return _module_cache[loop_n]


def kernel(x, adj=None):
    x = np.ascontiguousarray(np.asarray(x, dtype=np.float32))
    assert x.shape == (N, D)
    nc = _get_module(1)
    in_maps = [
        {"xf": x, "xi": x[c * ROWS : (c + 1) * ROWS]} for c in range(NCORES)
    ]
    res = run_bass_kernel_spmd(nc, in_maps, core_ids=list(range(NCORES)))
    return np.concatenate([res.results[c]["out"] for c in range(NCORES)], axis=0)



# revision 23
# speedup vs baseline: 1.0618x; 1.0618x over previous
"""BallQueryAttention TRN2 kernel (v6: single-pass fp16 Gram, block-outer).

Math: reference computes softmax over a binary ball mask (d2 <= R^2), then
mask-softmax @ x.  exp of a 0/1 mask takes only values {1, e}, so

  out[i] = (S + (e-1) * sum_{j in ball(i)} x_j) / (N + (e-1) * cnt_i)

with S = colsum(x).  Sharding: rows (i) across 8 cores, x replicated.

Numerics: a single fp16 Gram pass (hi_j . hi_i, PE accumulates fp32) gives
d2 errors ~6e-3 rms, flipping ~0.5 boundary points per row -> L2 rel err
~1.7e-3 on the softmax output, well under the 2e-2 gate (validated in
numpy against the fixed seed-0 input).

Per core (row shard of 1024):
  - Gram tiles Gt[j_tile(128 part), i(512 free)] via ONE fp16 matmul of
    67 contraction rows: stationary [hiT_j(64); ones(3)], moving
    [hiT_i(64); v1; v2; v3] where v* is a 3-term fp16 split of -0.5*sq_i.
    The sq_j term rides fp32 per-partition bias/threshold in the compare.
  - mask compare splits 29:35 between Vector (is_ge -> {0,2}) and Scalar
    (Sign -> {-1,1}) by j-tile, writing fp16 masks (ratio balances the
    two engines' per-op costs so both stay under the PE's tile budget).
  - accumulating [x|1]^T @ mask matmul -> [65, 512] per column block in
    PSUM, plus ones-column sums SALL (all tiles) and SAO (Sign tiles) to
    undo the mask conventions:  numer/denom = K1*OUT2 + (SALL + K1*SAO),
    K1 = (e-1)/2.
  - PE transpose + reciprocal + per-partition scale for the final divide.

Loop order is column-block-outer: all 64 j-tiles for i-columns 0:512,
then for 512:1024.  Block 0 finishes halfway through the kernel, so its
output tail overlaps block 1's main loop and the end bubble is halved.

Staging: all 16 x-group loads are issued up front into persistent SBUF
buffers (Sync HWDGE queue), so in-loop staging never waits on DMA.  HX
holds [x_fp16 | 1.0] rows: cols 0:65 of each 128-wide tile are the
pass-C stationary [x|1], and a DRAM round-trip + XBAR DMA-transpose of
the same bytes yields the Gram stationary [hiT_j; ones].  Pool does the
SBUF staging copies (it cannot touch PSUM on TRN2); Vector/Scalar hold
compares almost exclusively, so the PE never starves past the preamble.
"""

import sys

sys.path.insert(0, "/opt/trn_rl_repo")

import numpy as np

import concourse.bass as bass
import concourse.tile as tile
from concourse import bacc, masks, mybir
from concourse.bass_utils import run_bass_kernel_spmd

F32 = mybir.dt.float32
F16 = mybir.dt.float16
AF = mybir.ActivationFunctionType
OP = mybir.AluOpType

N = 8192
D = 64
NCORES = 8
ROWS = N // NCORES          # 1024 rows per core
JT = N // 128               # 64 j-tiles
IT = ROWS // 128            # 8 i-tiles
NG = 16                     # j-side column groups
TPG = JT // NG              # j-tiles per group
R2 = 11.0 * 11.0
K1 = (np.e - 1.0) / 2.0
KC = 67                     # Gram contraction: 64 hi dims + 3 aug rows
PRE = 4                     # groups staged before the main loop

# compare-engine split: DVE takes 29 of every 64 j-tiles (is_ge), ACT the
# other 35 (Sign); 29:35 equalizes the two engines' total busy time.
DVE_TILE = [((t * 29) % 64) < 29 for t in range(JT)]
ACT_TILES = [t for t in range(JT) if not DVE_TILE[t]]


def _stage_group_a(nc, pools, hilod, W1g, HX, g, xtp):
    """hi/ones staging into HX (Pool) + DRAM round-trip DMA transpose.

    HX holds [x_fp16(64) | 1.0(64)] per j-tile row: cols 0:65 of each tile
    are the pass-C stationary [x|1], and the XBAR transpose of the same
    bytes yields the Gram stationary [hiT_j; ones].  The xtp load was
    issued up front in _pre, so nothing here waits on input DMA."""
    const, scratch, gpool, mpool, apool, spool = pools
    rows = TPG * 128
    xtp3 = xtp[:].rearrange("p (t d) -> p t d", d=D)

    hx3 = HX[:].rearrange("p (t e) -> p t e", e=128)[:, g * TPG : (g + 1) * TPG, :]
    nc.gpsimd.tensor_copy(hx3[:, :, 0:D], xtp3)
    hseg = hilod[g * rows : (g + 1) * rows, :]
    nc.sync.dma_start(hseg.rearrange("(t p) e -> p t e", p=128),
                      HX[:, g * rows : (g + 1) * rows])
    nc.sync.dma_start(W1g[g][:], hseg, transpose=True)

    s2 = scratch.tile([128, TPG * D], F32, tag="s2")
    nc.scalar.activation(s2[:], xtp[:], AF.Square)
    return s2


def _stage_group_b(nc, pools, biasA, thrD, g, s2):
    """sq_j reduce (DVE) + threshold/bias derivation (Pool)."""
    sl = slice(g * TPG, (g + 1) * TPG)
    nc.vector.tensor_reduce(biasA[:, sl],
                            s2[:].rearrange("p (t d) -> p t d", d=D),
                            axis=mybir.AxisListType.X, op=OP.add)
    nc.gpsimd.tensor_scalar(thrD[:, sl], biasA[:, sl], 0.5, -R2 / 2.0,
                            OP.mult, OP.add)
    nc.gpsimd.tensor_scalar(biasA[:, sl], biasA[:, sl], -0.5, R2 / 2.0,
                            OP.mult, OP.add)


def _pre(nc, tc, pools, xf, xi, outd, dram):
    const, scratch, gpool, mpool, apool, spool = pools

    # ---------------- persistent tiles ----------------
    W1g = [const.tile([128, TPG * 128], F16, name=f"W1_{g}", tag=f"W1_{g}")
           for g in range(NG)]
    HX = const.tile([128, 128 * JT], F16, tag="HX")
    R1 = const.tile([128, ROWS], F16, tag="R1")     # [hiT_i; v1; v2; v3]
    biasA = const.tile([128, JT], F32, tag="biasA")
    thrD = const.tile([128, JT], F32, tag="thrD")
    ONEC = const.tile([128, 1], F16, tag="ONEC")
    IDN = const.tile([65, 65], F32, tag="IDN")

    r1d = dram.tile([ROWS, 128], F16, tag="r1d")
    hilod = dram.tile([N, 128], F16, tag="hilod")

    # constants + one-shot initializations first (no data deps)
    nc.vector.memset(ONEC[:], 1.0)
    hx_ones = HX[:].rearrange("p (t e) -> p t e", e=128)
    nc.gpsimd.memset(hx_ones[:, :, D:128], 1.0)    # aug rows + XBAR padding
    masks.make_identity(nc, IDN[:])
    # Sign act-table preload off a constant, so it can't delay the i side
    dumm = spool.tile([128, 1], F32, tag="dumm")
    nc.scalar.activation(dumm[:], ONEC[:], AF.Sign)

    # i-side load first: R1 is on the critical path to the first Gram
    xitp = scratch.tile([128, IT * D], F32, tag="xitp")  # row p*IT + t
    nc.sync.dma_start(xitp[:], xi.rearrange("(p t) d -> p (t d)", p=128))

    # all 16 group loads issued up front into persistent buffers, so
    # in-loop staging never waits on DMA.  j-tile t holds rows
    # {c*64+t : c in 0..127}: the load is contiguous per partition (the
    # j dimension is only ever summed over, so the relabeling is
    # invisible outside).
    xtps = [const.tile([128, TPG * D], F32, name=f"xtp{g}", tag=f"xtp{g}")
            for g in range(NG)]

    def load_group(g):
        nc.sync.dma_start(
            xtps[g][:].rearrange("p (t d) -> p t d", d=D),
            xf.rearrange("(p t) d -> p t d", p=128)[:, g * TPG : (g + 1) * TPG, :],
        )

    for g in range(NG):
        load_group(g)

    # ---------------- i side (small, DVE+ACT) ----------------
    xitp3 = xitp[:].rearrange("p (t d) -> p t d", d=D)
    r1sb = scratch.tile([128, IT * 128], F16, tag="r1sb")
    r13 = r1sb[:].rearrange("p (t e) -> p t e", e=128)
    nc.vector.tensor_copy(r13[:, :, 0:D], xitp3)          # hi_i
    nc.gpsimd.memset(r13[:, :, D + 3 : 128], 0.0)

    s2i = scratch.tile([128, IT * D], F32, tag="s2i")
    nc.scalar.activation(s2i[:], xitp[:], AF.Square)
    sqit = scratch.tile([128, IT], F32, tag="sqit")
    nc.vector.tensor_reduce(sqit[:], s2i[:].rearrange("p (t d) -> p t d", d=D),
                            axis=mybir.AxisListType.X, op=OP.add)
    vfull = scratch.tile([128, IT], F32, tag="vfull")
    nc.vector.tensor_scalar(vfull[:], sqit[:], -0.5, None, OP.mult)
    v1 = scratch.tile([128, IT], F16, tag="v1")
    nc.vector.tensor_copy(v1[:], vfull[:])
    rv1 = scratch.tile([128, IT], F32, tag="rv1")
    nc.vector.tensor_tensor(rv1[:], vfull[:], v1[:], OP.subtract)
    v2 = scratch.tile([128, IT], F16, tag="v2")
    nc.vector.tensor_copy(v2[:], rv1[:])
    rv2 = scratch.tile([128, IT], F32, tag="rv2")
    nc.vector.tensor_tensor(rv2[:], rv1[:], v2[:], OP.subtract)
    v3 = scratch.tile([128, IT], F16, tag="v3")
    nc.vector.tensor_copy(v3[:], rv2[:])
    for k, vk in enumerate((v1, v2, v3)):
        nc.vector.tensor_copy(
            r13[:, :, D + k : D + k + 1],
            vk[:].rearrange("p (t u) -> p t u", u=1),
        )
    nc.sync.dma_start(r1d[:].rearrange("(p t) e -> p (t e)", p=128), r1sb[:])
    nc.sync.dma_start(R1[:], r1d[:], transpose=True)

    # stage the lead groups ahead of the main loop
    s2s = {}
    for g in range(PRE):
        s2s[g] = _stage_group_a(nc, pools, hilod, W1g, HX, g, xtps[g])
        _stage_group_b(nc, pools, biasA, thrD, g, s2s.pop(g))

    return dict(W1g=W1g, HX=HX, R1=R1, biasA=biasA, thrD=thrD, ONEC=ONEC,
                IDN=IDN, hilod=hilod, s2s=s2s, xtps=xtps,
                load_group=load_group)


def _main(nc, tc, pools, outd, env):
    const, scratch, gpool, mpool, apool, spool = pools
    ts = bass.ts
    W1g = env['W1g']; HX = env['HX']; R1 = env['R1']
    biasA = env['biasA']; thrD = env['thrD']
    ONEC = env['ONEC']; IDN = env['IDN']
    hilod = env['hilod']; s2s = env['s2s']; xtps = env['xtps']

    # ---------------- psum accumulators ----------------
    # mask conventions by j-tile (DVE_TILE): DVE {0,2}, ACT {-1,1}.  With
    # SALL = sum[x|1] over all tiles and SAO over the Sign tiles:
    #   numer/denom = K1*OUT2 + (SALL + K1*SAO)
    OUT2 = apool.tile([65, ROWS], F32, tag="OUT2")
    SALL = apool.tile([65, 1], F32, tag="SALL")
    SAO = apool.tile([65, 1], F32, tag="SAO")

    # ------- block-outer main loop; pass C lagged by LAG steps ----------
    LAG = 3
    NH = 2 * JT
    mks = {}

    def tail_half(q):
        # numer/denom for i-cols [512q, 512q+512), transposed in 128-chunks
        pcW = spool.tile([65, 512], F32, tag="pcW")
        if q == 0:
            nc.vector.tensor_scalar(pcW[:], OUT2[:, 0:512], K1,
                                    env['base'][:], OP.mult, OP.add)
        else:
            nc.scalar.activation(pcW[:], OUT2[:, 512:1024], AF.Identity,
                                 bias=env['base'][:], scale=K1)
        pt4 = gpool.tile([128, 4 * 65], F32, tag="G")
        for u in range(4):
            nc.tensor.transpose(pt4[:, 65 * u : 65 * (u + 1)],
                                pcW[:, ts(u, 128)], IDN[:])
        pt43 = pt4[:].rearrange("p (u e) -> p u e", e=65)
        dinvW = spool.tile([128, 4], F32, tag="dinvW")
        nc.vector.reciprocal(dinvW[:], pt43[:, :, D : D + 1])
        otb = spool.tile([128, 4 * D], F32, tag="otb")
        for u in range(4):
            if u % 2 == 0:
                nc.vector.tensor_scalar(otb[:, ts(u, D)], pt43[:, u, 0:D],
                                        dinvW[:, u : u + 1], None, OP.mult)
            else:
                nc.scalar.activation(otb[:, ts(u, D)], pt43[:, u, 0:D],
                                     AF.Copy, scale=dinvW[:, u : u + 1])
        nc.sync.dma_start(
            outd[512 * q : 512 * (q + 1), :].rearrange("(u p) d -> p u d",
                                                       p=128),
            otb[:].rearrange("p (u d) -> p u d", d=D))

    for idx in range(NH + LAG):
        if idx < NH:
            h, t = divmod(idx, JT)
            g, tt = divmod(t, TPG)
            if h == 0:
                # interleaved j-side staging, PRE groups ahead
                if t % 4 == 0 and t // 4 + PRE < NG:
                    gs = t // 4 + PRE
                    s2s[gs] = _stage_group_a(nc, pools, hilod, W1g, HX, gs,
                                             xtps[gs])
                elif t % 4 == 2 and t // 4 + PRE < NG:
                    gs = t // 4 + PRE
                    _stage_group_b(nc, pools, biasA, thrD, gs, s2s.pop(gs))
            cs = slice(512 * h, 512 * (h + 1))
            Gh = gpool.tile([128, 512], F32, tag="G")
            nc.tensor.matmul(Gh[:], W1g[g][0:KC, ts(tt, 128)], R1[0:KC, cs],
                             start=True, stop=True)
            mk = mpool.tile([128, 512], F16, tag="mk")
            if DVE_TILE[t]:
                nc.vector.tensor_scalar(mk[:], Gh[:], thrD[:, t : t + 1],
                                        2.0, OP.is_ge, OP.mult)
            else:
                nc.scalar.activation(mk[:], Gh[:], AF.Sign,
                                     bias=biasA[:, t : t + 1])
            mks[idx] = mk
        if idx >= LAG:
            jdx = idx - LAG
            hj, tj = divmod(jdx, JT)
            cs = slice(512 * hj, 512 * (hj + 1))
            xws = HX[:, 128 * tj : 128 * tj + 65]
            nc.tensor.matmul(OUT2[:, cs], xws, mks.pop(jdx)[:],
                             start=(tj == 0), stop=(tj == JT - 1))
            if hj == 0:
                nc.tensor.matmul(SALL[:], xws, ONEC[:],
                                 start=(tj == 0), stop=(tj == JT - 1))
                if not DVE_TILE[tj]:
                    nc.tensor.matmul(SAO[:], xws, ONEC[:],
                                     start=(tj == ACT_TILES[0]),
                                     stop=(tj == ACT_TILES[-1]))
        # block-0 tail: base after block 0 completes, half during block 1
        if idx == JT + LAG:
            sallsb = spool.tile([65, 1], F32, tag="sallsb")
            nc.vector.tensor_copy(sallsb[:], SALL[:])
            base = spool.tile([65, 1], F32, tag="base")
            nc.vector.tensor_scalar(base[:], SAO[:], K1, sallsb[:],
                                    OP.mult, OP.add)
            env['base'] = base
        if idx == JT + LAG + 2:
            tail_half(0)

    tail_half(1)


def build_module(loop_n=1, scope='full'):
    nc = bacc.Bacc("TRN2", target_bir_lowering=False, debug=False,
                   num_devices=NCORES)
    xf_d = nc.dram_tensor("xf", [N, D], F32, kind="ExternalInput")
    xi_d = nc.dram_tensor("xi", [ROWS, D], F32, kind="ExternalInput")
    out_d = nc.dram_tensor("out", [ROWS, D], F32, kind="ExternalOutput")

    with tile.TileContext(nc) as tc:
        with (
            tc.tile_pool(name="const", bufs=1) as const,
            tc.tile_pool(name="scratch", bufs=2) as scratch,
            tc.tile_pool(name="gpool", bufs=4, space="PSUM") as gpool,
            tc.tile_pool(name="acc", bufs=1, space="PSUM") as apool,
            tc.tile_pool(name="mk", bufs=8) as mpool,
            tc.tile_pool(name="small", bufs=3) as spool,
            tc.tile_pool(name="dram", bufs=3, space="DRAM") as dram,
        ):
            pools = (const, scratch, gpool, mpool, apool, spool)
            args = (nc, tc, pools, xf_d.ap(), xi_d.ap(), out_d.ap(), dram)
            if loop_n == 1:
                env = _pre(*args)
                _main(nc, tc, pools, out_d.ap(), env)
            else:
                with tc.For_i(0, loop_n) as _:
                    env = _pre(*args)
                    _main(nc, tc, pools, out_d.ap(), env)
    nc.finalize()
    return nc


_module_cache = {}


def _get_module(loop_n=1):
    if loop_n not in _module_cache:
        _module_cache[loop_n] = build_module(loop_n)
    return _module_cache[loop_n]


def kernel(x, adj=None):
    x = np.ascontiguousarray(np.asarray(x, dtype=np.float32))
    assert x.shape == (N, D)
    nc = _get_module(1)
    in_maps = [
        {"xf": x, "xi": x[c * ROWS : (c + 1) * ROWS]} for c in range(NCORES)
    ]
    res = run_bass_kernel_spmd(nc, in_maps, core_ids=list(range(NCORES)))
    return np.concatenate([res.results[c]["out"] for c in range(NCORES)], axis=0)


# revision 32
# speedup vs baseline: 1.4705x; 1.3849x over previous
"""BallQueryAttention TRN2 kernel (v6: single-pass fp16 Gram, block-outer).

Math: reference computes softmax over a binary ball mask (d2 <= R^2), then
mask-softmax @ x.  exp of a 0/1 mask takes only values {1, e}, so

  out[i] = (S + (e-1) * sum_{j in ball(i)} x_j) / (N + (e-1) * cnt_i)

with S = colsum(x).  Sharding: rows (i) across 8 cores, x replicated.

Numerics: a single fp16 Gram pass (hi_j . hi_i, PE accumulates fp32) gives
d2 errors ~6e-3 rms, flipping ~0.5 boundary points per row -> L2 rel err
~1.7e-3 on the softmax output, well under the 2e-2 gate (validated in
numpy against the fixed seed-0 input).

Per core (row shard of 1024):
  - Gram tiles Gt[j_tile(128 part), i(512 free)] via ONE fp16 matmul of
    67 contraction rows: stationary [hiT_j(64); ones(3)], moving
    [hiT_i(64); v1; v2; v3] where v* is a 3-term fp16 split of -0.5*sq_i.
    The sq_j term rides fp32 per-partition bias/threshold in the compare.
  - mask compare splits 29:35 between Vector (is_ge -> {0,2}) and Scalar
    (Sign -> {-1,1}) by j-tile, writing fp16 masks (ratio balances the
    two engines' per-op costs so both stay under the PE's tile budget).
  - accumulating [x|1]^T @ mask matmul -> [65, 512] per column block in
    PSUM, plus ones-column sums SALL (all tiles) and SAO (Sign tiles) to
    undo the mask conventions:  numer/denom = K1*OUT2 + (SALL + K1*SAO),
    K1 = (e-1)/2.
  - PE transpose + reciprocal + per-partition scale for the final divide.

Loop order is column-block-outer: all 64 j-tiles for i-columns 0:512,
then for 512:1024.  Block 0 finishes halfway through the kernel, so its
output tail overlaps block 1's main loop and the end bubble is halved.

Staging: all 16 x-group loads are issued up front into persistent SBUF
buffers (Sync HWDGE queue), so in-loop staging never waits on DMA.  HX
holds [x_fp16 | 1.0] rows: cols 0:65 of each 128-wide tile are the
pass-C stationary [x|1], and a DRAM round-trip + XBAR DMA-transpose of
the same bytes yields the Gram stationary [hiT_j; ones].  Pool does the
SBUF staging copies (it cannot touch PSUM on TRN2); Vector/Scalar hold
compares almost exclusively, so the PE never starves past the preamble.
"""

import sys

sys.path.insert(0, "/opt/trn_rl_repo")

import numpy as np

import concourse.bass as bass
import concourse.tile as tile
from concourse import bacc, masks, mybir
from concourse.bass_utils import run_bass_kernel_spmd

F32 = mybir.dt.float32
F16 = mybir.dt.float16
AF = mybir.ActivationFunctionType
OP = mybir.AluOpType

N = 8192
D = 64
NCORES = 8
ROWS = N // NCORES          # 1024 rows per core
JT = N // 128               # 64 j-tiles
IT = ROWS // 128            # 8 i-tiles
NG = 16                     # j-side column groups
TPG = JT // NG              # j-tiles per group
R2 = 11.0 * 11.0
K1 = (np.e - 1.0) / 2.0
KC = 67                     # Gram contraction: 64 hi dims + 3 aug rows
PRE = 4                     # groups staged before the main loop



def _stage_group_a(nc, pools, hilod, W1g, HX, g, xtp):
    """hi/ones staging into HX (Pool) + DRAM round-trip DMA transpose.

    HX holds [x_fp16(64) | 1.0(64)] per j-tile row: cols 0:65 of each tile
    are the pass-C stationary [x|1], and the XBAR transpose of the same
    bytes yields the Gram stationary [hiT_j; ones].  The xtp load was
    issued up front in _pre, so nothing here waits on input DMA."""
    const, scratch, gpool, mpool, apool, spool, dbuf = pools
    rows = TPG * 128
    xtp3 = xtp[:].rearrange("p (t d) -> p t d", d=D)

    hx3 = HX[:].rearrange("p (t e) -> p t e", e=128)[:, g * TPG : (g + 1) * TPG, :]
    nc.gpsimd.tensor_copy(hx3[:, :, 0:D], xtp3)
    nc.gpsimd.memset(hx3[:, :, D : D + 4], 1.0)
    hseg = hilod[g * rows : (g + 1) * rows, :]
    nc.scalar.dma_start(hseg[:, 0:68].rearrange("(t p) e -> p t e", p=128),
                        HX[:].rearrange("p (t e) -> p t e", e=128)
                        [:, g * TPG : (g + 1) * TPG, 0:68])
    eng = nc.sync if g % 2 == 0 else nc.scalar
    eng.dma_start(W1g[g][:], hseg, transpose=True)

    s2 = scratch.tile([128, TPG * D], F32, tag="s2")
    nc.scalar.activation(s2[:], xtp[:], AF.Square)
    return s2


def _stage_group_b(nc, pools, biasA, thrD, g, s2):
    """sq_j reduce (DVE) + threshold/bias derivation (Pool)."""
    sl = slice(g * TPG, (g + 1) * TPG)
    nc.vector.tensor_reduce(biasA[:, sl],
                            s2[:].rearrange("p (t d) -> p t d", d=D),
                            axis=mybir.AxisListType.X, op=OP.add)
    nc.gpsimd.tensor_scalar(thrD[:, sl], biasA[:, sl], 0.5, -R2 / 2.0,
                            OP.mult, OP.add)
    nc.gpsimd.tensor_scalar(biasA[:, sl], biasA[:, sl], -0.5, R2 / 2.0,
                            OP.mult, OP.add)


def _pre(nc, tc, pools, xf, xi, outd, dram):
    const, scratch, gpool, mpool, apool, spool, dbuf = pools

    # ---------------- persistent tiles ----------------
    # W1g[g] / xtps[g] see their last read early in the main loop, so a
    # single buffer already overlaps across For_i iterations.  R1 / HX /
    # thrD / biasA are read until the loop's last tile, so they live in a
    # bufs=2 pool: iteration k+1 stages into the other buffer while
    # iteration k still runs.
    W1g = [const.tile([128, TPG * 128], F16, name=f"W1_{g}", tag=f"W1_{g}")
           for g in range(NG)]
    HX = dbuf.tile([128, 128 * JT], F16, tag="HX")
    R1 = dbuf.tile([128, ROWS], F16, tag="R1")      # [hiT_i; v1; v2; v3]
    biasA = dbuf.tile([128, JT], F32, tag="biasA")
    thrD = dbuf.tile([128, JT], F32, tag="thrD")
    ONEC = const.tile([128, 1], F16, tag="ONEC")
    IDN = const.tile([65, 65], F32, tag="IDN")

    r1d = dram.tile([ROWS, 128], F16, tag="r1d")
    hilod = dram.tile([N, 128], F16, tag="hilod")

    # constants + one-shot initializations first (no data deps)
    nc.vector.memset(ONEC[:], 1.0)
    masks.make_identity(nc, IDN[:])
    # Sign act-table preload off a constant, so it can't delay the i side
    dumm = spool.tile([128, 1], F32, tag="dumm")
    nc.scalar.activation(dumm[:], ONEC[:], AF.Sign)

    # i-side load first: R1 is on the critical path to the first Gram
    xitp = scratch.tile([128, IT * D], F32, tag="xitp")  # row p*IT + t
    nc.sync.dma_start(xitp[:], xi.rearrange("(p t) d -> p (t d)", p=128))

    # all 16 group loads issued up front into persistent buffers, so
    # in-loop staging never waits on DMA.  j-tile t holds rows
    # {c*64+t : c in 0..127}: the load is contiguous per partition (the
    # j dimension is only ever summed over, so the relabeling is
    # invisible outside).
    xtps = [const.tile([128, TPG * D], F32, name=f"xtp{g}", tag=f"xtp{g}")
            for g in range(NG)]

    def load_group(g):
        eng = nc.sync if g % 2 == 0 else nc.scalar
        eng.dma_start(
            xtps[g][:].rearrange("p (t d) -> p t d", d=D),
            xf.rearrange("(p t) d -> p t d", p=128)[:, g * TPG : (g + 1) * TPG, :],
        )

    for g in range(NG):
        load_group(g)

    # ---------------- i side (small, DVE+ACT) ----------------
    xitp3 = xitp[:].rearrange("p (t d) -> p t d", d=D)
    r1sb = scratch.tile([128, IT * 128], F16, tag="r1sb")
    r13 = r1sb[:].rearrange("p (t e) -> p t e", e=128)
    nc.vector.tensor_copy(r13[:, :, 0:D], xitp3)          # hi_i
    nc.gpsimd.memset(r13[:, :, D + 3 : 128], 0.0)

    s2i = scratch.tile([128, IT * D], F32, tag="s2i")
    nc.scalar.activation(s2i[:], xitp[:], AF.Square)
    sqit = scratch.tile([128, IT], F32, tag="sqit")
    nc.vector.tensor_reduce(sqit[:], s2i[:].rearrange("p (t d) -> p t d", d=D),
                            axis=mybir.AxisListType.X, op=OP.add)
    vfull = scratch.tile([128, IT], F32, tag="vfull")
    nc.vector.tensor_scalar(vfull[:], sqit[:], -0.5, None, OP.mult)
    v1 = scratch.tile([128, IT], F16, tag="v1")
    nc.vector.tensor_copy(v1[:], vfull[:])
    rv1 = scratch.tile([128, IT], F32, tag="rv1")
    nc.vector.tensor_tensor(rv1[:], vfull[:], v1[:], OP.subtract)
    v2 = scratch.tile([128, IT], F16, tag="v2")
    nc.vector.tensor_copy(v2[:], rv1[:])
    rv2 = scratch.tile([128, IT], F32, tag="rv2")
    nc.vector.tensor_tensor(rv2[:], rv1[:], v2[:], OP.subtract)
    v3 = scratch.tile([128, IT], F16, tag="v3")
    nc.vector.tensor_copy(v3[:], rv2[:])
    for k, vk in enumerate((v1, v2, v3)):
        nc.vector.tensor_copy(
            r13[:, :, D + k : D + k + 1],
            vk[:].rearrange("p (t u) -> p t u", u=1),
        )
    nc.scalar.dma_start(r1d[:].rearrange("(p t) e -> p (t e)", p=128), r1sb[:])
    nc.sync.dma_start(R1[:], r1d[:], transpose=True)

    # stage the lead groups ahead of the main loop
    s2s = {}
    for g in range(PRE):
        s2s[g] = _stage_group_a(nc, pools, hilod, W1g, HX, g, xtps[g])
        _stage_group_b(nc, pools, biasA, thrD, g, s2s.pop(g))

    return dict(W1g=W1g, HX=HX, R1=R1, biasA=biasA, thrD=thrD, ONEC=ONEC,
                IDN=IDN, hilod=hilod, s2s=s2s, xtps=xtps,
                load_group=load_group)


def _main(nc, tc, pools, outd, env):
    const, scratch, gpool, mpool, apool, spool, dbuf = pools
    ts = bass.ts
    W1g = env['W1g']; HX = env['HX']; R1 = env['R1']
    biasA = env['biasA']; thrD = env['thrD']
    ONEC = env['ONEC']; IDN = env['IDN']
    hilod = env['hilod']; s2s = env['s2s']; xtps = env['xtps']

    # ---------------- psum accumulators ----------------
    # i-column block 0 always gets the DVE {0,2} mask convention and
    # block 1 the ACT {-1,1} one, so the sign-correction term is just
    # SALL itself:
    #   numer/denom[:, 0:512]    = K1*OUT2 + SALL
    #   numer/denom[:, 512:1024] = K1*OUT2 + (1+K1)*SALL
    OUT2 = apool.tile([65, ROWS], F32, tag="OUT2")
    SALL = apool.tile([65, 1], F32, tag="SALL")

    # ---- main loop over j-tiles; both column blocks per tile so the
    # Gram pair shares one W1g ldweights and the pass-C pair (plus the
    # SALL ones-column) shares one HX ldweights.  pass C lags by LAG
    # tiles. ----
    LAG = 2
    mks = {}

    def tail_half(q, base):
        # numer/denom for i-cols [512q, 512q+512), transposed in 128-chunks
        pcW = spool.tile([65, 512], F32, name=f"pcW{q}", tag=f"pcW{q}")
        if q == 0:
            nc.vector.tensor_scalar(pcW[:], OUT2[:, 0:512], K1, base[:],
                                    OP.mult, OP.add)
        else:
            nc.scalar.activation(pcW[:], OUT2[:, 512:1024], AF.Identity,
                                 bias=base[:], scale=K1)
        pt4 = gpool.tile([128, 4 * 65], F32, tag="G")
        for u in range(4):
            nc.tensor.transpose(pt4[:, 65 * u : 65 * (u + 1)],
                                pcW[:, ts(u, 128)], IDN[:])
        pt43 = pt4[:].rearrange("p (u e) -> p u e", e=65)
        dinvW = spool.tile([128, 4], F32, name=f"dinvW{q}", tag=f"dinvW{q}")
        nc.vector.reciprocal(dinvW[:], pt43[:, :, D : D + 1])
        otb = spool.tile([128, 4 * D], F32, name=f"otb{q}", tag=f"otb{q}")
        for u in range(4):
            if u % 2 == 0:
                nc.vector.tensor_scalar(otb[:, ts(u, D)], pt43[:, u, 0:D],
                                        dinvW[:, u : u + 1], None, OP.mult)
            else:
                nc.scalar.activation(otb[:, ts(u, D)], pt43[:, u, 0:D],
                                     AF.Copy, scale=dinvW[:, u : u + 1])
        nc.sync.dma_start(
            outd[512 * q : 512 * (q + 1), :].rearrange("(u p) d -> p u d",
                                                       p=128),
            otb[:].rearrange("p (u d) -> p u d", d=D))

    for t in range(JT + LAG):
        if t < JT:
            g, tt = divmod(t, TPG)
            # interleaved j-side staging, PRE groups ahead
            if t % TPG == 0 and t // TPG + PRE < NG:
                gs = t // TPG + PRE
                s2s[gs] = _stage_group_a(nc, pools, hilod, W1g, HX, gs,
                                         xtps[gs])
            elif t % TPG == TPG // 2 and t // TPG + PRE < NG:
                gs = t // TPG + PRE
                _stage_group_b(nc, pools, biasA, thrD, gs, s2s.pop(gs))
            wsl = W1g[g][0:KC, ts(tt, 128)]
            Gh0 = gpool.tile([128, 512], F32, name="Gh0", tag="G")
            nc.tensor.matmul(Gh0[:], wsl, R1[0:KC, 0:512],
                             start=True, stop=True)
            Gh1 = gpool.tile([128, 512], F32, name="Gh1", tag="G")
            nc.tensor.matmul(Gh1[:], wsl, R1[0:KC, 512:1024],
                             start=True, stop=True)
            mk0 = mpool.tile([128, 512], F16, name="mk0", tag="mk")
            nc.vector.tensor_scalar(mk0[:], Gh0[:], thrD[:, t : t + 1],
                                    2.0, OP.is_ge, OP.mult)
            mk1 = mpool.tile([128, 512], F16, name="mk1", tag="mk")
            nc.scalar.activation(mk1[:], Gh1[:], AF.Sign,
                                 bias=biasA[:, t : t + 1])
            mks[t] = (mk0, mk1)
        if t >= LAG:
            tj = t - LAG
            xws = HX[:, 128 * tj : 128 * tj + 65]
            m0, m1 = mks.pop(tj)
            nc.tensor.matmul(OUT2[:, 0:512], xws, m0[:],
                             start=(tj == 0), stop=(tj == JT - 1))
            nc.tensor.matmul(OUT2[:, 512:1024], xws, m1[:],
                             start=(tj == 0), stop=(tj == JT - 1))
            nc.tensor.matmul(SALL[:], xws, ONEC[:],
                             start=(tj == 0), stop=(tj == JT - 1))

    # ---------------- tail ----------------
    sallsb = spool.tile([65, 1], F32, tag="sallsb")
    nc.vector.tensor_copy(sallsb[:], SALL[:])
    b1sb = spool.tile([65, 1], F32, tag="b1sb")
    nc.vector.tensor_scalar(b1sb[:], sallsb[:], 1.0 + K1, None, OP.mult)
    tail_half(0, sallsb)
    tail_half(1, b1sb)


def build_module(loop_n=1, scope='full'):
    nc = bacc.Bacc("TRN2", target_bir_lowering=False, debug=False,
                   num_devices=NCORES)
    xf_d = nc.dram_tensor("xf", [N, D], F32, kind="ExternalInput")
    xi_d = nc.dram_tensor("xi", [ROWS, D], F32, kind="ExternalInput")
    out_d = nc.dram_tensor("out", [ROWS, D], F32, kind="ExternalOutput")

    with tile.TileContext(nc) as tc:
        with (
            tc.tile_pool(name="const", bufs=1) as const,
            tc.tile_pool(name="scratch", bufs=2) as scratch,
            tc.tile_pool(name="gpool", bufs=4, space="PSUM") as gpool,
            tc.tile_pool(name="acc", bufs=1, space="PSUM") as apool,
            tc.tile_pool(name="mk", bufs=8) as mpool,
            tc.tile_pool(name="small", bufs=3) as spool,
            tc.tile_pool(name="dbuf", bufs=2) as dbuf,
            tc.tile_pool(name="dram", bufs=3, space="DRAM") as dram,
        ):
            pools = (const, scratch, gpool, mpool, apool, spool, dbuf)
            args = (nc, tc, pools, xf_d.ap(), xi_d.ap(), out_d.ap(), dram)
            if scope == 'pre':
                with tc.For_i(0, loop_n) as _:
                    _pre(*args)
            elif scope == 'main':
                env = _pre(*args)
                with tc.For_i(0, loop_n) as _:
                    _main(nc, tc, pools, out_d.ap(), env)
            elif loop_n == 1:
                env = _pre(*args)
                _main(nc, tc, pools, out_d.ap(), env)
            else:
                with tc.For_i(0, loop_n) as _:
                    env = _pre(*args)
                    _main(nc, tc, pools, out_d.ap(), env)
    nc.finalize()
    return nc


_module_cache = {}


def _get_module(loop_n=1):
    if loop_n not in _module_cache:
        _module_cache[loop_n] = build_module(loop_n)
    return _module_cache[loop_n]


def kernel(x, adj=None):
    x = np.ascontiguousarray(np.asarray(x, dtype=np.float32))
    assert x.shape == (N, D)
    nc = _get_module(1)
    in_maps = [
        {"xf": x, "xi": x[c * ROWS : (c + 1) * ROWS]} for c in range(NCORES)
    ]
    res = run_bass_kernel_spmd(nc, in_maps, core_ids=list(range(NCORES)))
    return np.concatenate([res.results[c]["out"] for c in range(NCORES)], axis=0)
